# revision 24
# baseline (speedup 1.0000x reference)
"""Trainium2 Bass kernel for the char-LSTM word-similarity CNN scorer.

Problem: B=8192 examples x NW=4 words x L=16 chars. Per word: char
embeddings -> masked LSTMCell over <=16 steps -> cell state c [128].
Per example: 4x4 cosine matrix of the word reps -> 2-layer 2x2-valid
CNN -> linear scorer -> sigmoid.

Strategy (pure data parallel, 1024 examples / 4096 words per core):
 - Host folds emb @ W_ih.T + (b_ih + b_hh) into a [66, 512] table G65
   with gate-column order (i, f, o, g); per-step char inputs become a
   K=66 one-hot matmul (row 64 = "freeze" flag driving f->1, i->0 for
   words past their length, so no masking/select ops on device).
 - Words sorted by length (desc) on host; step t processes exactly
   W[t] columns (max alive over cores, rounded to 16) in <=512-col
   chunks. Gate PSUM layout [i|f|o|g] at 512-col strides lets ONE wide
   Sigmoid ACTIVATE cover i,f,o (amortizing the ~352-cycle fixed cost);
   tanh(g)/tanh(c) are separate. sigma(o)/tanh(c)/h only computed on
   the next-step-alive prefix. Gate activations are bf16 so DVE
   tensor_tensor ops hit 2x mode where both operands are 16-bit.
 - Streamed tail: when a 128-col group of sorted words freezes
   (host-known step), cast c->bf16 (DVE copy), DMA-xbar transpose
   (idle DMA queue; no PSUM), indirect-scatter rows to DRAM in
   example-grouped order (idle GpSimd) - all hidden under the LSTM.
 - Post-loop: one strided readback -> A [128 ex-part, (word, h)],
   norms via square+reduce+ln/exp, 6 pair mul+reduce dots, rsqrt-norm
   scaling on the tiny [128, 8] dot tiles, PE transposes to [6, 1024],
   then the 2x2 convs + scorer as tiny host-built matmuls.
"""

import os
import sys

for _p in ("/opt/trn_rl_repo",):
    if _p not in sys.path and os.path.isdir(_p):
        sys.path.insert(0, _p)

import ml_dtypes
import numpy as np

import concourse.bass as bass
import concourse.mybir as mybir
import concourse.tile as tile
from concourse.bass_utils import run_bass_kernel_spmd
from concourse.masks import make_identity

# This container's walrus build rejects CTRL instructions (Drain) carrying
# more than 2 sync waits ("Too many sync wait commands" in setupSyncWait).
# Tile's kernel-tail drain accumulates one wait per engine/DMA-queue sem, so
# redistribute: keep one wait on the drain, move the rest onto nofuse NOPs
# that execute before the all-engine barrier. Semantics are unchanged (all
# waits still complete before the barrier / semaphore teardown).
def _patched_drain_and_barrier(self, tick_clock, wait_clock):
    nc = self.nc
    drain_inst = nc.sync.drain()
    wait_clock.add_sem_waits(
        drain_inst.ins, tile.ScopedClock({None: tick_clock.global_clock})
    )
    waits = list(drain_inst.ins.sync_info.on_wait)
    if len(waits) > 1:
        drain_inst.ins.sync_info.on_wait = waits[:1]
        for k in range(1, len(waits)):
            nop = nc.sync.nop(nofuse=True, hint="drain_wait_spill")
            if nop.ins.sync_info is None:
                nop.ins.sync_info = mybir.SyncInfo(on_wait=[], on_update=[])
            nop.ins.sync_info.on_wait = [waits[k]]
    nc.all_engine_barrier()
    assert self.sems is not None
    popped = nc._tile_sem_poison_stack.pop()
    assert popped is self._sem_poison
    nc.clear_and_free_semaphores(list(self.sems.allocated().values()))
    nc.all_engine_barrier()


tile.TileContext._drain_and_barrier = _patched_drain_and_barrier

def _spill_excess_waits(nc):
    """Walrus here rejects instructions with more than ~2 sync waits. Spill
    excess waits onto same-engine NoOps inserted just before the instruction
    (engines dispatch in program order, so waiting earlier on the same engine
    is equivalent)."""
    cnt = [0]
    for fn in nc.m.functions:
        for bb in fn.blocks:
            insts = list(bb.instructions)
            out = []
            changed = False
            for inst in insts:
                si = inst.sync_info
                waits = list(si.on_wait) if si is not None and si.on_wait else []
                max_waits = 1
                if len(waits) > max_waits:
                    changed = True
                    keep = waits[-max_waits:]
                    extra = waits[:-max_waits]
                    for j in range(0, len(extra), max_waits):
                        cnt[0] += 1
                        nop = mybir.InstNoOp(name=f"I-spillw-{cnt[0]}", ins=[], outs=[])
                        nop.engine = inst.engine
                        nop.sync_info = mybir.SyncInfo(
                            on_wait=extra[j:j + max_waits], on_update=[])
                        nop.bass_nofuse = True
                        nop.bass_priority = 0
                        nop.text_hint = "spillw"
                        nop.debug = inst.debug
                        out.append(nop)
                    si.on_wait = keep
                out.append(inst)
            if changed:
                bb.instructions = out

B, NW, L, E, H, V = 8192, 4, 16, 128, 128, 64
NCORES = 8
PER = B // NCORES          # 1024 examples per core
NWORD = PER * NW           # 4096 words per core
NEC = PER // 128           # 8 example-chunks of 128
BLK = 512                  # words per PSUM chunk
NG = NWORD // 128          # 32 groups of 128 sorted words
FB = 30.0                  # freeze bias magnitude
F32 = mybir.dt.float32
BF16 = mybir.dt.bfloat16
I32 = mybir.dt.int32
AF = mybir.ActivationFunctionType
ALU = mybir.AluOpType

P6 = [(0, 1), (0, 2), (0, 3), (1, 2), (1, 3), (2, 3)]


# ----------------------------------------------------------------- host prep

def _build_consts(inp):
    emb = np.asarray(inp["emb_i"], np.float32)
    W_ih = np.asarray(inp["W_ih"], np.float32)
    W_hh = np.asarray(inp["W_hh"], np.float32)
    b = np.asarray(inp["b_ih"], np.float32) + np.asarray(inp["b_hh"], np.float32)
    # gate-column reorder (torch i,f,g,o) -> (i,f,o,g)
    gorder = np.r_[0:H, H:2 * H, 3 * H:4 * H, 2 * H:3 * H]
    G = np.zeros((V + 2, 4 * H), np.float32)
    G[:V] = (emb @ W_ih.T + b)[:, gorder]
    G[V, 0:H] = -FB            # i -> 0
    G[V, H:2 * H] = +FB        # f -> 1
    WhhT = np.ascontiguousarray(W_hh.T[:, gorder])

    w1 = np.asarray(inp["conv1_w"], np.float32)
    b1 = np.asarray(inp["conv1_b"], np.float32)
    w2 = np.asarray(inp["conv2_w"], np.float32)
    b2 = np.asarray(inp["conv2_b"], np.float32)
    ws = np.asarray(inp["scorer_w"], np.float32)
    bs = float(np.asarray(inp["scorer_b"], np.float32)[0])

    p6idx = {p: i for i, p in enumerate(P6)}
    W1eff = np.zeros((6, 36), np.float32)
    b1eff = np.zeros((36, 1), np.float32)
    for c in range(4):
        for y in range(3):
            for x in range(3):
                m = c * 9 + y * 3 + x
                b1eff[m, 0] += b1[c]
                for dy in range(2):
                    for dx in range(2):
                        a, bb = y + dy, x + dx
                        w = w1[c, 0, dy, dx]
                        if a == bb:
                            b1eff[m, 0] += w
                        else:
                            W1eff[p6idx[(min(a, bb), max(a, bb))], m] += w
    W2eff = np.zeros((36, 32), np.float32)
    b2eff = np.zeros((32, 1), np.float32)
    for c2 in range(8):
        for y in range(2):
            for x in range(2):
                m = c2 * 4 + y * 2 + x
                b2eff[m, 0] = b2[c2]
                for c1 in range(4):
                    for dy in range(2):
                        for dx in range(2):
                            W2eff[c1 * 9 + (y + dy) * 3 + (x + dx), m] += w2[c2, c1, dy, dx]
    Wsc = ws[0].astype(np.float32).reshape(32, 1)
    return dict(G65=G, WhhT=WhhT, W1eff=W1eff, b1eff=b1eff,
                W2eff=W2eff, b2eff=b2eff, Wsc=Wsc, bsc=bs)


def _core_prep(word_ids_c, lengths_c):
    wid = np.asarray(word_ids_c).reshape(NWORD, L)
    lens = np.asarray(lengths_c).reshape(NWORD)
    perm = np.argsort(-lens, kind="stable")
    wid_s = wid[perm]
    lens_s = lens[perm]
    Nt = (np.arange(L)[:, None] < lens_s[None, :]).sum(1)  # alive count per step
    # scatter destination row (example-grouped layout) per sorted position
    e = perm // NW
    i = perm % NW
    dest = (i * PER + e).astype(np.int32)          # [NWORD]
    idx = np.ascontiguousarray(dest.reshape(NG, 128).T)  # [128, NG]
    return wid_s, lens_s, Nt, idx


def _build_onehot(wid_s, lens_s, W, off, tot):
    oh = np.zeros((V + 2, tot), np.float32)
    for t in range(L):
        n = int(W[t])
        if n == 0:
            continue
        ch = np.where(lens_s[:n] > t, wid_s[:n, t], V)
        oh[ch, off[t] + np.arange(n)] = 1.0
    return oh


# -------------------------------------------------------------- bass program

def _schedule(W):
    """W: per-step widths. Returns (off, tot, groups_by_t)."""
    off = np.zeros(L, np.int64)
    for t in range(1, L):
        off[t] = off[t - 1] + W[t - 1]
    tot = int(off[-1] + W[-1])
    # group g (cols [128g,128g+128)) finalizes after the last step with W > 128g
    groups_by_t = {t: [] for t in range(L)}
    for g in range(NG):
        fg = max(t for t in range(L) if W[t] > 128 * g)
        groups_by_t[fg].append(g)
    return off, tot, groups_by_t


def _build_program(W):
    W = list(W) + [0]
    off, tot, groups_by_t = _schedule(W[:L])

    nc = bass.Bass()
    oh_in = nc.dram_tensor("oh", [V + 2, tot], BF16, kind="ExternalInput")
    idx_in = nc.dram_tensor("idx", [128, NG], I32, kind="ExternalInput")
    g65_in = nc.dram_tensor("g65", [V + 2, 4 * H], BF16, kind="ExternalInput")
    whht_in = nc.dram_tensor("whht", [H, 4 * H], BF16, kind="ExternalInput")
    w1_in = nc.dram_tensor("w1", [6, 36], BF16, kind="ExternalInput")
    b1_in = nc.dram_tensor("b1", [36, 1], F32, kind="ExternalInput")
    w2_in = nc.dram_tensor("w2", [36, 32], BF16, kind="ExternalInput")
    b2_in = nc.dram_tensor("b2", [32, 1], F32, kind="ExternalInput")
    wsc_in = nc.dram_tensor("wsc", [32, 1], BF16, kind="ExternalInput")
    bsc_in = nc.dram_tensor("bsc", [1, 1], F32, kind="ExternalInput")
    out_d = nc.dram_tensor("out", [1, PER], F32, kind="ExternalOutput")
    e_dram = nc.dram_tensor("escratch", [NWORD, H], BF16)

    with tile.TileContext(nc) as tc:
        with (
            tc.tile_pool(name="const", bufs=1) as cpool,
            tc.tile_pool(name="state", bufs=1) as spool,
        ):
            g65_sb = cpool.tile([V + 2, 4 * H], BF16, tag="g65", name="g65")
            whht_sb = cpool.tile([H, 4 * H], BF16, tag="whht", name="whht")
            idx_sb = cpool.tile([128, NG], I32, tag="idx", name="idx")
            w1_sb = cpool.tile([6, 36], BF16, tag="w1", name="w1")
            b1_sb = cpool.tile([36, 1], F32, tag="b1", name="b1")
            w2_sb = cpool.tile([36, 32], BF16, tag="w2", name="w2")
            b2_sb = cpool.tile([32, 1], F32, tag="b2", name="b2")
            wsc_sb = cpool.tile([32, 1], BF16, tag="wsc", name="wsc")
            bsc_sb = cpool.tile([1, 1], F32, tag="bsc", name="bsc")
            ident = cpool.tile([128, 128], F32, tag="ident", name="ident")
            for sb, dr in ((g65_sb, g65_in), (whht_sb, whht_in)):
                nc.sync.dma_start(sb[:], dr[:])
            make_identity(nc, ident[:])

            NBLK = (max(W[:L]) + BLK - 1) // BLK
            c_blk = [spool.tile([H, BLK], BF16, tag=f"c{k}", name=f"c{k}")
                     for k in range(NBLK)]
            h_blk = [spool.tile([H, BLK], BF16, tag=f"h{k}", name=f"h{k}")
                     for k in range(NBLK)]

            # warm the PE clock gate while the first DMAs land
            with tc.tile_pool(name="warm", bufs=1, space="PSUM") as wpsum:
                wp = wpsum.tile([128, 128], F32, tag="wp", name="wp")
                for _ in range(20):
                    nc.tensor.transpose(wp[:], ident[:], ident[:])

            # ------------------------------------------------ LSTM main loop
            with (
                tc.tile_pool(name="oh", bufs=2) as ohpool,
                tc.tile_pool(name="gates", bufs=2, space="PSUM") as gpsum,
                tc.tile_pool(name="act", bufs=3) as apool,
                tc.tile_pool(name="scat", bufs=6) as scpool,
            ):
                oh_tiles = {}
                def load_oh(t, split=False):
                    if t >= L or W[t] == 0 or t in oh_tiles:
                        return
                    sb = ohpool.tile([V + 2, NWORD], BF16, tag="oh", name="oh")
                    o0 = int(off[t])
                    if split:
                        cut = min(2 * BLK, W[t])
                        nc.sync.dma_start(sb[:, :cut], oh_in[:, o0:o0 + cut])
                        if W[t] > cut:
                            nc.sync.dma_start(sb[:, cut:W[t]],
                                              oh_in[:, o0 + cut:o0 + W[t]])
                    else:
                        nc.sync.dma_start(sb[:, :W[t]], oh_in[:, o0:o0 + W[t]])
                    oh_tiles[t] = sb

                pending = []
                def flush_pending(full=True):
                    while len(pending) > (0 if full else 1):
                        pk, pwn, pifo = pending.pop(0)
                        tct = apool.tile([128, BLK], BF16, tag="tc", name="tc")
                        nc.scalar.activation(tct[:, :pwn], c_blk[pk][:, :pwn],
                                             AF.Tanh)
                        nc.vector.tensor_mul(h_blk[pk][:, :pwn],
                                             pifo[:, 2 * BLK:2 * BLK + pwn],
                                             tct[:, :pwn])

                load_oh(0, split=True)
                # tail-only constants: issue after the hot-path DMAs
                for sb, dr in ((idx_sb, idx_in), (w1_sb, w1_in), (b1_sb, b1_in),
                               (w2_sb, w2_in), (b2_sb, b2_in), (wsc_sb, wsc_in),
                               (bsc_sb, bsc_in)):
                    nc.sync.dma_start(sb[:], dr[:])
                for t in range(L):
                    Wt = W[t]
                    if Wt == 0:
                        continue
                    ct = (Wt + BLK - 1) // BLK
                    oh_sb = oh_tiles.pop(t)
                    for k in range(ct):
                        w = min(BLK, Wt - BLK * k)
                        wn = max(0, min(W[t + 1] - BLK * k, w))  # next-alive prefix
                        P = gpsum.tile([128, 4 * BLK], F32, tag="gates", name="gates")
                        # sigma gates (i,f,o) first so the wide sigmoid can
                        # start before the g matmuls finish
                        for m in (0, 1, 2):
                            nc.tensor.matmul(
                                P[:, BLK * m:BLK * m + w],
                                lhsT=g65_sb[:, H * m:H * (m + 1)],
                                rhs=oh_sb[:, BLK * k:BLK * k + w],
                                start=True, stop=(t == 0))
                        if t > 0:
                            for m in (0, 1, 2):
                                nc.tensor.matmul(
                                    P[:, BLK * m:BLK * m + w],
                                    lhsT=whht_sb[:, H * m:H * (m + 1)],
                                    rhs=h_blk[k][:, :w],
                                    start=False, stop=True)
                        nc.tensor.matmul(
                            P[:, 3 * BLK:3 * BLK + w],
                            lhsT=g65_sb[:, 3 * H:4 * H],
                            rhs=oh_sb[:, BLK * k:BLK * k + w],
                            start=True, stop=(t == 0))
                        if t > 0:
                            nc.tensor.matmul(
                                P[:, 3 * BLK:3 * BLK + w],
                                lhsT=whht_sb[:, 3 * H:4 * H],
                                rhs=h_blk[k][:, :w],
                                start=False, stop=True)
                        if k == 0:
                            load_oh(t + 1)  # prefetch ahead of freeze-group DMAs
                        ifo = apool.tile([128, 3 * BLK], BF16, tag="ifo", name="ifo")
                        nc.scalar.activation(ifo[:, :2 * BLK + wn],
                                             P[:, :2 * BLK + wn], AF.Sigmoid)
                        gt = apool.tile([128, BLK], BF16, tag="g", name="g")
                        nc.scalar.activation(gt[:, :w], P[:, 3 * BLK:3 * BLK + w],
                                             AF.Tanh)
                        flush_pending(full=False)
                        if t == 0:
                            nc.vector.tensor_mul(c_blk[k][:, :w],
                                                 ifo[:, :w], gt[:, :w])
                        else:
                            u = apool.tile([128, BLK], BF16, tag="u", name="u")
                            nc.vector.tensor_mul(u[:, :w], ifo[:, :w], gt[:, :w])
                            nc.vector.tensor_mul(c_blk[k][:, :w],
                                                 ifo[:, BLK:BLK + w],
                                                 c_blk[k][:, :w])
                            nc.vector.tensor_add(c_blk[k][:, :w],
                                                 c_blk[k][:, :w], u[:, :w])
                        if wn > 0:
                            pending.append((k, wn, ifo))
                    flush_pending()
                    # finalize groups whose last alive step was t: cast ->
                    # xbar transpose -> indirect scatter (example layout)
                    for g in groups_by_t[t]:
                        blk, rel = g // 4, (g % 4) * 128
                        tb = scpool.tile([128, 128], BF16, tag="tb", name="tb")
                        nc.sync.dma_start_transpose(
                            tb[:], c_blk[blk][:, rel:rel + 128])
                        nc.gpsimd.indirect_dma_start(
                            out=e_dram[:],
                            out_offset=bass.IndirectOffsetOnAxis(
                                ap=idx_sb[:, g:g + 1], axis=0),
                            in_=tb[:],
                            in_offset=None,
                        )

                # gather-back + norms + pair dots INSIDE the loop pools so the
                # pool-teardown drains overlap them; per-quarter squares start
                # as each readback DMA lands
                A = spool.tile([128, NWORD], BF16, tag="A", name="A")
                d_all = spool.tile([128, NW * NEC], F32, tag="d", name="d")
                rn = spool.tile([128, NW * NEC], F32, tag="rn", name="rn")
                Dp = spool.tile([128, 6 * NEC], F32, tag="Dp", name="Dp")
                rnp = spool.tile([128, 6 * NEC], F32, tag="rnp", name="rnp")
                sqscr = spool.tile([128, 128], F32, tag="sqscr", name="sqscr")
                NB4 = NW * NEC // 4
                for q in range(4):
                    dq = nc.scalar if q % 2 else nc.sync
                    dq.dma_start(
                        A[:, q * NB4 * H:(q + 1) * NB4 * H].rearrange(
                            "p (b h) -> p b h", b=NB4),
                        e_dram[q * NB4 * 128:(q + 1) * NB4 * 128, :].rearrange(
                            "(b p) h -> p b h", p=128))
                    if q < 2:
                        for b in range(q * NB4, (q + 1) * NB4):
                            nc.scalar.activation(sqscr[:], A[:, b * H:(b + 1) * H],
                                                 AF.Square,
                                                 accum_out=d_all[:, b:b + 1])
                asq = spool.tile([128, 2 * NB4 * H], BF16, tag="asq", name="asq")
                half = 2 * NB4 * H
                nc.vector.tensor_mul(asq[:], A[:, half:], A[:, half:])
                nc.vector.tensor_reduce(
                    d_all[:, 2 * NB4:], asq[:].rearrange("p (b h) -> p b h",
                                                         b=2 * NB4),
                    axis=mybir.AxisListType.X, op=ALU.add)
                nc.vector.tensor_scalar_max(d_all[:], d_all[:], 1e-30)
                nc.scalar.activation(rn[:], d_all[:], AF.Ln)
                nc.scalar.activation(rn[:], rn[:], AF.Exp, scale=-0.5)
                for k, (i, j) in enumerate(P6):
                    scr = apool.tile([128, NEC * 128], BF16, tag="scr", name="scr")
                    nc.vector.tensor_mul(
                        scr[:, :NEC * 128], A[:, i * PER:(i + 1) * PER],
                        A[:, j * PER:(j + 1) * PER])
                    nc.vector.tensor_reduce(
                        Dp[:, k * NEC:(k + 1) * NEC],
                        scr[:, :NEC * 128].rearrange("p (e h) -> p e h", e=NEC),
                        axis=mybir.AxisListType.X, op=ALU.add)
                    nc.vector.tensor_mul(rnp[:, k * NEC:(k + 1) * NEC],
                                         rn[:, i * NEC:(i + 1) * NEC],
                                         rn[:, j * NEC:(j + 1) * NEC])
                nc.vector.tensor_mul(Dp[:], Dp[:], rnp[:])

            # ------------------------------------------------------- tail
            with (
                tc.tile_pool(name="big", bufs=1) as big,
                tc.tile_pool(name="tpsum", bufs=2, space="PSUM") as tpsum,
                tc.tile_pool(name="cpsum", bufs=1, space="PSUM") as cpsum,
                tc.tile_pool(name="small", bufs=1) as small,
                tc.tile_pool(name="scr", bufs=2) as scrp,
            ):

                cos6 = small.tile([6, PER], BF16, tag="cos6", name="cos6")
                for ec in range(NEC):
                    pt = tpsum.tile([128, 128], F32, tag="tp", name="tp")
                    dview = bass.AP(Dp.tensor, Dp.offset + ec,
                                    [Dp.ap[0], [NEC, 6]])
                    nc.tensor.transpose(pt[:6, :], dview, ident[:])
                    nc.vector.tensor_copy(cos6[:, ec * 128:(ec + 1) * 128], pt[:6, :])

                r1 = small.tile([36, PER], BF16, tag="r1", name="r1")
                r2 = small.tile([32, PER], BF16, tag="r2", name="r2")
                o_sb = small.tile([1, PER], F32, tag="o", name="o")
                p1 = cpsum.tile([36, PER], F32, tag="cp1", name="cp1")
                p2 = cpsum.tile([32, PER], F32, tag="cp2", name="cp2")
                p3 = cpsum.tile([1, PER], F32, tag="cp3", name="cp3")
                for half in range(2):
                    sl = slice(half * 512, (half + 1) * 512)
                    nc.tensor.matmul(p1[:, sl], lhsT=w1_sb[:], rhs=cos6[:, sl],
                                     start=True, stop=True)
                    nc.scalar.activation(r1[:, sl], p1[:, sl], AF.Relu,
                                         bias=b1_sb[:, 0:1])
                    nc.tensor.matmul(p2[:, sl], lhsT=w2_sb[:], rhs=r1[:, sl],
                                     start=True, stop=True)
                    nc.scalar.activation(r2[:, sl], p2[:, sl], AF.Relu,
                                         bias=b2_sb[:, 0:1])
                    nc.tensor.matmul(p3[:, sl], lhsT=wsc_sb[:], rhs=r2[:, sl],
                                     start=True, stop=True)
                    nc.scalar.activation(o_sb[:, sl], p3[:, sl], AF.Sigmoid,
                                         bias=bsc_sb[0:1, 0:1])
                nc.sync.dma_start(out_d[:], o_sb[:])

    return nc


_prog_cache = {}


def _get_program(W):
    key = tuple(int(x) for x in W)
    if key not in _prog_cache:
        _prog_cache[key] = _build_program(key)
    return _prog_cache[key]


def _run(inputs, trace=False):
    consts = _build_consts(inputs)
    word_ids = np.asarray(inputs["word_ids"])
    lengths = np.asarray(inputs["lengths"])

    preps = []
    for c in range(NCORES):
        sl = slice(c * PER, (c + 1) * PER)
        preps.append(_core_prep(word_ids[sl], lengths[sl]))
    Nt_max = np.stack([p[2] for p in preps]).max(0)
    W = tuple(int(min(NWORD, -(-int(n) // 16) * 16)) for n in Nt_max)
    off, tot, _ = _schedule(list(W))

    g65_bf = consts["G65"].astype(ml_dtypes.bfloat16)
    whht_bf = consts["WhhT"].astype(ml_dtypes.bfloat16)
    in_maps = []
    for c in range(NCORES):
        wid_s, lens_s, _, idx = preps[c]
        in_maps.append({
            "oh": _build_onehot(wid_s, lens_s, W, off, tot).astype(ml_dtypes.bfloat16),
            "idx": idx,
            "g65": g65_bf, "whht": whht_bf,
            "w1": consts["W1eff"].astype(ml_dtypes.bfloat16), "b1": consts["b1eff"],
            "w2": consts["W2eff"].astype(ml_dtypes.bfloat16), "b2": consts["b2eff"],
            "wsc": consts["Wsc"].astype(ml_dtypes.bfloat16),
            "bsc": np.full((1, 1), consts["bsc"], np.float32),
        })

    nc = _get_program(W)
    _spill_excess_waits(nc)  # idempotent; HW-compile only
    res = run_bass_kernel_spmd(nc, in_maps, list(range(NCORES)), trace=trace)
    out = np.concatenate([np.asarray(r["out"]).reshape(PER) for r in res.results])
    return out.reshape(B, 1).astype(np.float32), res.exec_time_ns


def kernel(**inputs):
    return _run(inputs)[0]
